# revision 1
# baseline (speedup 1.0000x reference)
"""Trainium2 Bass kernel for KeypointAlignmentLossL2.

Strategy (data-parallel over batch, one NeuronCore per batch element):
  Host prep (per core b):
    - transpose feat[b] from [C, H*W] to pixel-major [H*W, C], cast bf16
    - compute bilinear corner indices / weights from kp[b] (f32, exact
      floor/sub semantics; x0 clamped to W-2 with wx in [0,1] so all four
      corners are always in-bounds — identical math to the reference's
      zero-padded gather for coords in [0, W-1])
    - weights are packed as 128x128 bf16 diagonal matrices so the lerp can
      run on the tensor engine as accumulating diagonal matmuls
  Device (per core):
    - dma_gather: 4 corner rows (768 ch, bf16) per keypoint straight from
      HBM into SBUF, keypoint -> partition
    - TensorE: f = sum_nb diag(w_nb) @ g_nb accumulated in PSUM (f32)
    - ScalarE: copy f PSUM->SBUF
    - VectorE: fused tensor_tensor_reduce for ||f1||^2, ||f2||^2, <f1,f2>
    - outputs three [128, 8] f32 tiles (keypoint-chunk layout)
  Host finish: masked mean of 2 - 2*cos distances across all cores.
"""
import numpy as np
import ml_dtypes

B, C, H, W, N = 8, 768, 64, 64, 1024
HW_ = H * W
NCHUNK = N // 128  # 8 chunks of 128 keypoints
NQ = 4             # gather calls per image; each covers 2 chunks (1024 idxs)

_CACHE = {}


def _build_nc():
    from contextlib import ExitStack
    import concourse.bass as bass
    import concourse.tile as tile
    import concourse.mybir as mybir
    from concourse import bacc

    f32 = mybir.dt.float32
    bf16 = mybir.dt.bfloat16
    i16 = mybir.dt.int16

    nc = bacc.Bacc("TRN2", target_bir_lowering=False, debug=False, num_devices=8)

    featT1 = nc.dram_tensor("featT1", [HW_, C], bf16, kind="ExternalInput")
    featT2 = nc.dram_tensor("featT2", [HW_, C], bf16, kind="ExternalInput")
    idx1 = nc.dram_tensor("idx1", [128, 4 * N // 16], i16, kind="ExternalInput")
    idx2 = nc.dram_tensor("idx2", [128, 4 * N // 16], i16, kind="ExternalInput")
    wd = nc.dram_tensor("wd", [128, 2 * NCHUNK * 4, 128], bf16, kind="ExternalInput")
    out_n1 = nc.dram_tensor("out_n1", [128, NCHUNK], f32, kind="ExternalOutput")
    out_n2 = nc.dram_tensor("out_n2", [128, NCHUNK], f32, kind="ExternalOutput")
    out_dot = nc.dram_tensor("out_dot", [128, NCHUNK], f32, kind="ExternalOutput")

    featTs = (featT1, featT2)
    idxs_dram = (idx1, idx2)
    MULT = mybir.AluOpType.mult
    ADD = mybir.AluOpType.add

    with tile.TileContext(nc) as tc, ExitStack() as ctx:
        const_pool = ctx.enter_context(tc.tile_pool(name="const", bufs=1))
        gpool = ctx.enter_context(tc.tile_pool(name="g", bufs=4))
        fpool = ctx.enter_context(tc.tile_pool(name="f", bufs=4))
        dpool = ctx.enter_context(tc.tile_pool(name="d", bufs=2))
        ppool = ctx.enter_context(
            tc.tile_pool(name="p", bufs=8, space=bass.MemorySpace.PSUM)
        )

        wd_t = const_pool.tile([128, 2 * NCHUNK * 4, 128], bf16, tag="wd")
        nc.sync.dma_start(wd_t[:], wd[:])
        idx_t = []
        for im in range(2):
            t = const_pool.tile([128, 4 * N // 16], i16, tag=f"idx{im}", name=f"idx{im}")
            nc.sync.dma_start(t[:], idxs_dram[im][:])
            idx_t.append(t)

        res = []
        for name in ("n1", "n2", "dot"):
            res.append(const_pool.tile([128, NCHUNK], f32, tag=f"res_{name}", name=f"res_{name}"))

        for q in range(NQ):
            gt = []
            for im in range(2):
                g = gpool.tile([128, 2 * 4, C], bf16, tag="g")
                nc.gpsimd.dma_gather(
                    g[:],
                    featTs[im][:],
                    idx_t[im][:, q * 64:(q + 1) * 64],
                    1024,
                    1024,
                    C,
                )
                gt.append(g)
            for j in range(2):
                ch = 2 * q + j
                fs = []
                for im in range(2):
                    f_sb = fpool.tile([128, C], f32, tag="f")
                    for h in range(2):
                        ps = ppool.tile([128, C // 2], f32, tag="ps")
                        for nb in range(4):
                            nc.tensor.matmul(
                                ps[:],
                                wd_t[:, (im * NCHUNK + ch) * 4 + nb, :],
                                gt[im][:, 4 * j + nb, h * (C // 2):(h + 1) * (C // 2)],
                                start=(nb == 0),
                                stop=(nb == 3),
                            )
                        nc.scalar.copy(f_sb[:, h * (C // 2):(h + 1) * (C // 2)], ps[:])
                    fs.append(f_sb)
                dump_a = dpool.tile([128, C], f32, tag="dump_a", name="dump_a")
                dump_b = dpool.tile([128, C], f32, tag="dump_b", name="dump_b")
                nc.scalar.activation(
                    dump_a[:], fs[0][:], mybir.ActivationFunctionType.Square,
                    accum_out=res[0][:, ch:ch + 1],
                )
                nc.scalar.activation(
                    dump_a[:], fs[1][:], mybir.ActivationFunctionType.Square,
                    accum_out=res[1][:, ch:ch + 1],
                )
                nc.vector.tensor_tensor(dump_b[:], fs[0][:], fs[1][:], op=MULT)
                nc.vector.tensor_reduce(
                    res[2][:, ch:ch + 1], dump_b[:],
                    axis=mybir.AxisListType.X, op=ADD,
                )

        nc.sync.dma_start(out_n1[:], res[0][:])
        nc.sync.dma_start(out_n2[:], res[1][:])
        nc.sync.dma_start(out_dot[:], res[2][:])

    nc.compile()
    return nc


def get_nc():
    if "nc" not in _CACHE:
        _CACHE["nc"] = _build_nc()
    return _CACHE["nc"]


def _host_prep_img(feat_b, kp_b):
    """feat_b [C,H,W] f32, kp_b [N,2] f32 ->
    featT bf16 [HW_, C], nb_idx int32 [4, N], w f32 [4, N]"""
    featT = np.ascontiguousarray(
        np.asarray(feat_b, np.float32).reshape(C, HW_).T
    ).astype(ml_dtypes.bfloat16)
    x = np.asarray(kp_b[:, 0], np.float32)
    y = np.asarray(kp_b[:, 1], np.float32)
    x0 = np.minimum(np.floor(x), np.float32(W - 2)).astype(np.float32)
    y0 = np.minimum(np.floor(y), np.float32(H - 2)).astype(np.float32)
    wx = (x - x0).astype(np.float32)
    wy = (y - y0).astype(np.float32)
    pix = y0.astype(np.int32) * W + x0.astype(np.int32)
    nb_idx = np.stack([pix, pix + 1, pix + W, pix + W + 1], 0)
    w = np.stack(
        [(1 - wx) * (1 - wy), wx * (1 - wy), (1 - wx) * wy, wx * wy], 0
    ).astype(np.float32)
    return featT, nb_idx, w


def _make_idx_layout(nb_idx):
    """[4,N] corner indices -> [128, 4N/16] int16 SBUF index layout
    (element i=(4*ch+nb)*128+p lives at [i%16 (replicated x8), i//16])."""
    unwrapped = nb_idx.reshape(4, NCHUNK, 128).transpose(1, 0, 2).reshape(-1)
    lay = unwrapped.reshape(-1, 16).T
    return np.tile(lay, (8, 1)).astype(np.int16)


def _make_wd(w1, w2):
    """weights [4,N] f32 per image -> [128, 64, 128] bf16 diagonal matrices"""
    wd = np.zeros((128, 2 * NCHUNK * 4, 128), np.float32)
    r = np.arange(128)
    for im, w in ((0, w1), (1, w2)):
        for ch in range(NCHUNK):
            for nb in range(4):
                k = (im * NCHUNK + ch) * 4 + nb
                wd[r, k, r] = w[nb, ch * 128:(ch + 1) * 128]
    return wd.astype(ml_dtypes.bfloat16)


def kernel(feat1, feat2, kp1, kp2, kp1_mask, kp2_mask):
    from concourse.bass_utils import run_bass_kernel_spmd

    feat1 = np.asarray(feat1, np.float32)
    feat2 = np.asarray(feat2, np.float32)
    kp1 = np.asarray(kp1, np.float32)
    kp2 = np.asarray(kp2, np.float32)
    kp1_mask = np.asarray(kp1_mask)
    kp2_mask = np.asarray(kp2_mask)

    nc = get_nc()
    in_maps = []
    for b in range(B):
        fT1, nb1, w1 = _host_prep_img(feat1[b], kp1[b])
        fT2, nb2, w2 = _host_prep_img(feat2[b], kp2[b])
        in_maps.append({
            "featT1": fT1,
            "featT2": fT2,
            "idx1": _make_idx_layout(nb1),
            "idx2": _make_idx_layout(nb2),
            "wd": _make_wd(w1, w2),
        })

    results = run_bass_kernel_spmd(nc, in_maps, list(range(B))).results

    sum_l2 = 0.0
    sum_valid = 0.0
    for b in range(B):
        r = results[b]
        n1sq = r["out_n1"].T.reshape(-1).astype(np.float64)
        n2sq = r["out_n2"].T.reshape(-1).astype(np.float64)
        dot = r["out_dot"].T.reshape(-1).astype(np.float64)
        m1 = np.maximum(np.sqrt(n1sq), 1e-12)
        m2 = np.maximum(np.sqrt(n2sq), 1e-12)
        l2 = n1sq / (m1 * m1) + n2sq / (m2 * m2) - 2.0 * dot / (m1 * m2)
        valid = (kp1_mask[b] & kp2_mask[b]).astype(np.float64)
        sum_l2 += float((l2 * valid).sum())
        sum_valid += float(valid.sum())

    loss = 0.0 if sum_valid == 0 else sum_l2 / max(sum_valid, 1.0)
    return np.float32(loss)



# revision 12
# speedup vs baseline: 1.2591x; 1.2591x over previous
"""Trainium2 Bass kernel for KeypointAlignmentLossL2.

Strategy (data-parallel over batch, one NeuronCore per batch element):
  Host prep (per core b):
    - transpose feat[b] from [C, H*W] to pixel-major [H*W, C], cast fp8-e4m3
    - compute bilinear corner indices / weights from kp[b] (f32, exact
      floor/sub semantics; x0/y0 clamped to W-2 with wx in [0,1] so all four
      corners are always in-bounds -- identical math to the reference's
      zero-padded gather for coords in [0, W-1])
    - weights packed as 128x128 fp8 diagonal matrices so the lerp runs on
      the tensor engine as accumulating diagonal matmuls
  Device (per core):
    - dma_gather (SWDGE prepare_only + trigger_dma so the 8 gathers pipeline
      back-to-back on the DMA engines): each descriptor fetches a horizontal
      PIXEL PAIR (x0,x0+1) = 1536 B fp8, two descriptors per keypoint (y0/y1
      rows), keypoint -> partition
    - TensorE: f = sum_nb diag(w_nb) @ g_nb accumulated in PSUM (f32),
      fp8 matmuls, N=384 halves
    - VectorE: copy f2 PSUM->SBUF bf16; fused tensor_tensor_reduce for
      dot = sum(f1*f2)
    - ScalarE: activation(Square, accum_out) for |f1|^2 and |f2|^2
    - outputs one [128, 24] f32 tile (n1 | n2 | dot, keypoint-chunk layout)
  Host finish: masked mean of 2 - 2*cos distances across all cores.
"""
import copy as _pycopy
import numpy as np
import ml_dtypes

B, C, H, W, N = 8, 768, 64, 64, 1024
HW_ = H * W
NCHUNK = N // 128  # 8 chunks of 128 keypoints
NQ = 4             # gather calls per image; each covers 2 chunks (512 idxs)

_CACHE = {}


def _overlap_ap(dram_handle):
    """Flat [HW_*C] fp8 dram tensor -> AP [[C, HW_-2], [1, 2*C]] so that
    dma_gather with elem_step=C and elem_size=2*C fetches a horizontal
    pixel pair per index (idx in pixel units)."""
    import bass_rust
    base = dram_handle[:].rearrange("(r c) -> r c", c=2 * C)
    ap = _pycopy.copy(base)
    ap.ap = bass_rust.VecI64Pair([[C, HW_ - 1], [1, 2 * C]])
    return ap


def _build_nc():
    from contextlib import ExitStack
    import concourse.bass as bass
    import concourse.tile as tile
    import concourse.mybir as mybir
    from concourse import bacc

    f32 = mybir.dt.float32
    bf16 = mybir.dt.bfloat16
    fp8 = mybir.dt.float8e4
    i16 = mybir.dt.int16
    MULT = mybir.AluOpType.mult
    ADD = mybir.AluOpType.add
    SQUARE = mybir.ActivationFunctionType.Square

    nc = bacc.Bacc("TRN2", target_bir_lowering=False, debug=False, num_devices=8)

    featT1 = nc.dram_tensor("featT1", [HW_ * C], fp8, kind="ExternalInput")
    featT2 = nc.dram_tensor("featT2", [HW_ * C], fp8, kind="ExternalInput")
    idx1 = nc.dram_tensor("idx1", [128, 2 * N // 16], i16, kind="ExternalInput")
    idx2 = nc.dram_tensor("idx2", [128, 2 * N // 16], i16, kind="ExternalInput")
    wd = nc.dram_tensor("wd", [128, 2 * NCHUNK * 4, 128], fp8, kind="ExternalInput")
    out_res = nc.dram_tensor("out_res", [128, 3 * NCHUNK], f32, kind="ExternalOutput")

    feat_aps = (_overlap_ap(featT1), _overlap_ap(featT2))
    idxs_dram = (idx1, idx2)

    from concourse.tile_scheduler import PROC_NAME_TO_IDX

    with tile.TileContext(nc) as tc, ExitStack() as ctx:
        const_pool = ctx.enter_context(tc.tile_pool(name="const", bufs=1))
        f2c_pool = ctx.enter_context(tc.tile_pool(name="f2c", bufs=3))
        dump_pool = ctx.enter_context(tc.tile_pool(name="dump", bufs=6))
        ppool = ctx.enter_context(
            tc.tile_pool(name="p", bufs=4, space=bass.MemorySpace.PSUM)
        )

        wd_t = const_pool.tile([128, 2 * NCHUNK * 4, 128], fp8, tag="wd")
        nc.sync.dma_start(wd_t[:], wd[:])
        idx_t = []
        for im in range(2):
            t = const_pool.tile([128, 2 * N // 16], i16, tag=f"idx{im}", name=f"idx{im}")
            nc.sync.dma_start(t[:], idxs_dram[im][:])
            idx_t.append(t)

        # res layout: cols [0:8] = |f1|^2, [8:16] = |f2|^2, [16:24] = dot
        res = const_pool.tile([128, 3 * NCHUNK], f32, tag="res", name="res")

        # All gather outputs preallocated; preps+triggers issued up-front so
        # the SWDGE ring drains continuously.
        g_tiles = [[None] * NQ for _ in range(2)]
        gsems = []
        for q in range(NQ):
            for im in range(2):
                g = const_pool.tile(
                    [128, 4, 2 * C], fp8, tag=f"g{im}_{q}", name=f"g{im}_{q}"
                )
                g_tiles[im][q] = g
                sem = nc.alloc_semaphore(f"gsem_{im}_{q}")
                gsems.append(sem)
                nc.gpsimd.dma_gather(
                    g[:],
                    feat_aps[im],
                    idx_t[im][:, q * 32:(q + 1) * 32],
                    512,
                    512,
                    2 * C,
                    elem_step=C,
                    prepare_only=True,
                    sem=sem,
                )
                nc.gpsimd.trigger_dma(count=None)

        for q in range(NQ):
            for j in range(2):
                ch = 2 * q + j
                # --- TensorE: bilinear lerp into PSUM, per image ---
                ps = []
                for im in range(2):
                    p = ppool.tile([128, 1024], f32, tag="ps")
                    # h0 -> [:, 128:512] (end of bank 0), h1 -> [:, 512:896]
                    # (start of bank 1): each matmul output sits inside one
                    # PSUM bank, but [:, 128:896] is a contiguous 2D region
                    # so each downstream reduction is a single pass.
                    for h in range(2):
                        for nb in range(4):
                            k = (im * NCHUNK + ch) * 4 + nb
                            mm = nc.tensor.matmul(
                                p[:, 128 + 384 * h:512 + 384 * h],
                                wd_t[:, k, :],
                                g_tiles[im][q][
                                    :, 2 * j + (nb >> 1),
                                    (nb & 1) * C + h * 384:(nb & 1) * C + h * 384 + 384,
                                ],
                                start=(nb == 0),
                                stop=(nb == 3),
                            )
                            if nb == 0:
                                # Gate each accumulation group on the gather's
                                # DMA-completion sem; tile's prepare_only path
                                # does not auto-gate on-chip consumers.
                                mm._wait_ge(gsems[q * 2 + im], 16)
                    ps.append(p)
                f1_ap = ps[0][:, 128:896]
                f2_ap = ps[1][:, 128:896]

                # --- VectorE: f2 PSUM->SBUF bf16, then fused dot ---
                f2c = f2c_pool.tile([128, 768], bf16, tag="f2c")
                nc.vector.tensor_copy(f2c[:], f2_ap)
                dump_d = dump_pool.tile([128, 768], bf16, tag="dump_d", name="dump_d")
                nc.vector.scalar_tensor_tensor(
                    dump_d[:], f1_ap, 1.0, f2c[:], MULT, MULT,
                    accum_out=res[:, 16 + ch:16 + ch + 1],
                )

                # --- ScalarE: |f1|^2 and |f2|^2 ---
                dump_a = dump_pool.tile([128, 768], bf16, tag="dump_a", name="dump_a")
                dump_b = dump_pool.tile([128, 768], bf16, tag="dump_b", name="dump_b")
                nc.scalar.activation(
                    dump_a[:], f1_ap, SQUARE, accum_out=res[:, ch:ch + 1]
                )
                nc.scalar.activation(
                    dump_b[:], f2c[:], SQUARE, accum_out=res[:, 8 + ch:8 + ch + 1]
                )

        nc.sync.dma_start(out_res[:], res[:])

    nc.compile()
    return nc


def get_nc():
    if "nc" not in _CACHE:
        _CACHE["nc"] = _build_nc()
    return _CACHE["nc"]


def _host_prep_img(feat_b, kp_b):
    """feat_b [C,H,W] f32, kp_b [N,2] f32 ->
    featT fp8 flat [HW_*C], iy int32 [2, N] (pixel idx of y0/y1 row starts),
    w f32 [4, N]"""
    featT = np.ascontiguousarray(
        np.asarray(feat_b, np.float32).reshape(C, HW_).T
    ).astype(ml_dtypes.float8_e4m3).reshape(-1)
    x = np.asarray(kp_b[:, 0], np.float32)
    y = np.asarray(kp_b[:, 1], np.float32)
    x0 = np.minimum(np.floor(x), np.float32(W - 2)).astype(np.float32)
    y0 = np.minimum(np.floor(y), np.float32(H - 2)).astype(np.float32)
    wx = (x - x0).astype(np.float32)
    wy = (y - y0).astype(np.float32)
    pix = y0.astype(np.int32) * W + x0.astype(np.int32)
    iy = np.stack([pix, pix + W], 0)  # descriptor start pixels (y0 / y1 rows)
    w = np.stack(
        [(1 - wx) * (1 - wy), wx * (1 - wy), (1 - wx) * wy, wx * wy], 0
    ).astype(np.float32)
    return featT, iy, w


def _make_idx_layout(iy):
    """[2,N] y-row start pixels -> [128, 2N/16] int16 SBUF index layout.
    Sequence order: per gather call q (512 idxs): ranks = [chunk 2q y0,
    chunk 2q y1, chunk 2q+1 y0, chunk 2q+1 y1], 128 kps each. Wrapped so
    sequence element i lives at [i%16 (replicated x8), i//16]."""
    seq = np.empty(2 * N, np.int32)
    pos = 0
    for q in range(NQ):
        for j in range(2):
            chunk = 2 * q + j
            sl = slice(chunk * 128, (chunk + 1) * 128)
            seq[pos:pos + 128] = iy[0, sl]
            seq[pos + 128:pos + 256] = iy[1, sl]
            pos += 256
    lay = seq.reshape(-1, 16).T
    return np.tile(lay, (8, 1)).astype(np.int16)


def _make_wd(w1, w2):
    """weights [4,N] f32 per image -> [128, 64, 128] fp8 diagonal matrices"""
    wd = np.zeros((128, 2 * NCHUNK * 4, 128), np.float32)
    r = np.arange(128)
    for im, w in ((0, w1), (1, w2)):
        for ch in range(NCHUNK):
            for nb in range(4):
                k = (im * NCHUNK + ch) * 4 + nb
                wd[r, k, r] = w[nb, ch * 128:(ch + 1) * 128]
    return wd.astype(ml_dtypes.float8_e4m3)


def build_in_maps(feat1, feat2, kp1, kp2):
    in_maps = []
    for b in range(B):
        fT1, iy1, w1 = _host_prep_img(feat1[b], kp1[b])
        fT2, iy2, w2 = _host_prep_img(feat2[b], kp2[b])
        in_maps.append({
            "featT1": fT1,
            "featT2": fT2,
            "idx1": _make_idx_layout(iy1),
            "idx2": _make_idx_layout(iy2),
            "wd": _make_wd(w1, w2),
        })
    return in_maps


def kernel(feat1, feat2, kp1, kp2, kp1_mask, kp2_mask):
    from concourse.bass_utils import run_bass_kernel_spmd

    feat1 = np.asarray(feat1, np.float32)
    feat2 = np.asarray(feat2, np.float32)
    kp1 = np.asarray(kp1, np.float32)
    kp2 = np.asarray(kp2, np.float32)
    kp1_mask = np.asarray(kp1_mask)
    kp2_mask = np.asarray(kp2_mask)

    nc = get_nc()
    in_maps = build_in_maps(feat1, feat2, kp1, kp2)
    results = run_bass_kernel_spmd(nc, in_maps, list(range(B))).results

    sum_l2 = 0.0
    sum_valid = 0.0
    for b in range(B):
        r = results[b]["out_res"]
        n1sq = r[:, 0:8].T.reshape(-1).astype(np.float64)
        n2sq = r[:, 8:16].T.reshape(-1).astype(np.float64)
        dot = r[:, 16:24].T.reshape(-1).astype(np.float64)
        m1 = np.maximum(np.sqrt(n1sq), 1e-12)
        m2 = np.maximum(np.sqrt(n2sq), 1e-12)
        l2 = n1sq / (m1 * m1) + n2sq / (m2 * m2) - 2.0 * dot / (m1 * m2)
        valid = (kp1_mask[b] & kp2_mask[b]).astype(np.float64)
        sum_l2 += float((l2 * valid).sum())
        sum_valid += float(valid.sum())

    loss = 0.0 if sum_valid == 0 else sum_l2 / max(sum_valid, 1.0)
    return np.float32(loss)


# revision 13
# speedup vs baseline: 1.5740x; 1.2501x over previous
"""Trainium2 Bass kernel for KeypointAlignmentLossL2.

Strategy (data-parallel over batch, one NeuronCore per batch element):
  Host prep (per core b):
    - repack feat[b] into a pair-interleaved pixel-major fp8 layout
      ("featPair"): part A = rows (0,1),(2,3),... interleaved per column,
      part B = rows (1,2),(3,4),... . A keypoint's whole 2x2 bilinear patch
      is then 3072 contiguous bytes at a single host-computed index
      (part A for even y0, part B for odd y0) -> ONE gather descriptor per
      keypoint (SWDGE desc-gen at ~8 ns/desc is the pool-engine bottleneck).
    - bilinear weights packed as 128x128 fp8 diagonal matrices so the lerp
      runs on the tensor engine as accumulating diagonal matmuls
  Device (per core):
    - dma_gather (SWDGE prepare_only + trigger_dma so gathers pipeline
      back-to-back on the DMA engines), keypoint -> partition
    - TensorE: f = sum_nb diag(w_nb) @ g_nb accumulated in PSUM (f32),
      fp8 matmuls; the two N=384 halves land at psum[:, 128:512] and
      [:, 512:896] (each inside one bank, contiguous as a read region)
    - VectorE: copy f2 PSUM->SBUF bf16; scalar_tensor_tensor computes
      dot = sum(f1*f2) in one fused pass
    - ScalarE: activation(Square, accum_out) for |f1|^2 and |f2|^2
    - outputs one [128, 24] f32 tile (n1 | n2 | dot, keypoint-chunk layout)
  Host finish: masked mean of 2 - 2*cos distances across all cores.
"""
import copy as _pycopy
import numpy as np
import ml_dtypes

B, C, H, W, N = 8, 768, 64, 64, 1024
HW_ = H * W
NCHUNK = N // 128   # 8 chunks of 128 keypoints
NQ = 4              # gather calls per image; each covers 2 chunks (256 idxs)
NPAIR_A = HW_ // 2            # 2048 pair-slots in part A (even y0)
NPAIR_B = (H - 2) // 2 * W    # 1984 pair-slots in part B (odd y0)
NPAIR = NPAIR_A + NPAIR_B

_CACHE = {}


def _pair_ap(dram_handle):
    """Flat [NPAIR*1536] fp8 dram tensor -> AP [[1536, NPAIR-1], [1, 3072]]
    so dma_gather with elem_step=1536 and elem_size=3072 fetches a 2x2
    pixel patch per index (idx in pair-slot units)."""
    import bass_rust
    base = dram_handle[:].rearrange("(r c) -> r c", c=3072)
    ap = _pycopy.copy(base)
    ap.ap = bass_rust.VecI64Pair([[1536, NPAIR - 1], [1, 3072]])
    return ap


def _build_nc():
    from contextlib import ExitStack
    import concourse.bass as bass
    import concourse.tile as tile
    import concourse.mybir as mybir
    from concourse import bacc

    f32 = mybir.dt.float32
    bf16 = mybir.dt.bfloat16
    fp8 = mybir.dt.float8e4
    i16 = mybir.dt.int16
    MULT = mybir.AluOpType.mult
    SQUARE = mybir.ActivationFunctionType.Square

    nc = bacc.Bacc("TRN2", target_bir_lowering=False, debug=False, num_devices=8)

    featP1 = nc.dram_tensor("featP1", [NPAIR * 2 * C], fp8, kind="ExternalInput")
    featP2 = nc.dram_tensor("featP2", [NPAIR * 2 * C], fp8, kind="ExternalInput")
    idx1 = nc.dram_tensor("idx1", [128, N // 16], i16, kind="ExternalInput")
    idx2 = nc.dram_tensor("idx2", [128, N // 16], i16, kind="ExternalInput")
    wd = nc.dram_tensor("wd", [128, 2 * NCHUNK * 4, 128], fp8, kind="ExternalInput")
    out_res = nc.dram_tensor("out_res", [128, 3 * NCHUNK], f32, kind="ExternalOutput")

    feat_aps = (_pair_ap(featP1), _pair_ap(featP2))
    idxs_dram = (idx1, idx2)
    # corner nb (reference order: y0x0, y0x1, y1x0, y1x1) -> byte offset in
    # the gathered pair-interleaved patch [y0x0 | y1x0 | y0x1 | y1x1]
    CORNER_OFF = (0, 2 * C, C, 3 * C)

    with tile.TileContext(nc) as tc, ExitStack() as ctx:
        const_pool = ctx.enter_context(tc.tile_pool(name="const", bufs=1))
        f2c_pool = ctx.enter_context(tc.tile_pool(name="f2c", bufs=3))
        dump_pool = ctx.enter_context(tc.tile_pool(name="dump", bufs=6))
        ppool = ctx.enter_context(
            tc.tile_pool(name="p", bufs=4, space=bass.MemorySpace.PSUM)
        )

        # Explicit zero bias for activations: keeps bass's const-AP database
        # (pool-engine memsets) out of the kernel so the GPSIMD library load
        # for dma_gather starts immediately after the entry barrier.
        zbias = const_pool.tile([128, 1], f32, tag="zbias", name="zbias")
        nc.vector.memset(zbias[:], 0.0)

        wd_t = const_pool.tile([128, 2 * NCHUNK * 4, 128], fp8, tag="wd")
        nc.sync.dma_start(wd_t[:], wd[:])
        idx_t = []
        for im in range(2):
            t = const_pool.tile([128, N // 16], i16, tag=f"idx{im}", name=f"idx{im}")
            nc.sync.dma_start(t[:], idxs_dram[im][:])
            idx_t.append(t)

        # res layout: cols [0:8] = |f1|^2, [8:16] = |f2|^2, [16:24] = dot
        res = const_pool.tile([128, 3 * NCHUNK], f32, tag="res", name="res")

        # All gather outputs preallocated; preps+triggers issued up-front so
        # the SWDGE ring drains continuously.
        g_tiles = [[None] * NQ for _ in range(2)]
        gsems = []
        for q in range(NQ):
            for im in range(2):
                g = const_pool.tile(
                    [128, 2, 4 * C], fp8, tag=f"g{im}_{q}", name=f"g{im}_{q}"
                )
                g_tiles[im][q] = g
                sem = nc.alloc_semaphore(f"gsem_{im}_{q}")
                gsems.append(sem)
                nc.gpsimd.dma_gather(
                    g[:],
                    feat_aps[im],
                    idx_t[im][:, q * 16:(q + 1) * 16],
                    256,
                    256,
                    4 * C,
                    elem_step=2 * C,
                    prepare_only=True,
                    sem=sem,
                )
                nc.gpsimd.trigger_dma(count=None)

        for q in range(NQ):
            for j in range(2):
                ch = 2 * q + j
                # --- TensorE: bilinear lerp into PSUM, per image ---
                ps = []
                for im in range(2):
                    p = ppool.tile([128, 1024], f32, tag="ps")
                    for h in range(2):
                        for nb in range(4):
                            k = (im * NCHUNK + ch) * 4 + nb
                            off = CORNER_OFF[nb] + h * 384
                            mm = nc.tensor.matmul(
                                p[:, 128 + 384 * h:512 + 384 * h],
                                wd_t[:, k, :],
                                g_tiles[im][q][:, j, off:off + 384],
                                start=(nb == 0),
                                stop=(nb == 3),
                            )
                            if nb == 0:
                                # Gate each accumulation group on the gather's
                                # DMA-completion sem; tile's prepare_only path
                                # does not auto-gate on-chip consumers.
                                mm._wait_ge(gsems[q * 2 + im], 16)
                    ps.append(p)
                f1_ap = ps[0][:, 128:896]
                f2_ap = ps[1][:, 128:896]

                # --- VectorE: f2 PSUM->SBUF bf16, then fused dot ---
                f2c = f2c_pool.tile([128, 768], bf16, tag="f2c")
                nc.vector.tensor_copy(f2c[:], f2_ap)
                dump_d = dump_pool.tile([128, 768], bf16, tag="dump_d", name="dump_d")
                nc.vector.scalar_tensor_tensor(
                    dump_d[:], f1_ap, 1.0, f2c[:], MULT, MULT,
                    accum_out=res[:, 16 + ch:16 + ch + 1],
                )

                # --- ScalarE: |f1|^2 and |f2|^2 ---
                dump_a = dump_pool.tile([128, 768], bf16, tag="dump_a", name="dump_a")
                dump_b = dump_pool.tile([128, 768], bf16, tag="dump_b", name="dump_b")
                nc.scalar.activation(
                    dump_a[:], f1_ap, SQUARE, bias=zbias[:],
                    accum_out=res[:, ch:ch + 1],
                )
                nc.scalar.activation(
                    dump_b[:], f2c[:], SQUARE, bias=zbias[:],
                    accum_out=res[:, 8 + ch:8 + ch + 1],
                )

        nc.sync.dma_start(out_res[:], res[:])

    nc.compile()
    return nc


def get_nc():
    if "nc" not in _CACHE:
        _CACHE["nc"] = _build_nc()
    return _CACHE["nc"]


def _host_prep_img(feat_b, kp_b):
    """feat_b [C,H,W] f32, kp_b [N,2] f32 ->
    featPair fp8 flat [NPAIR*1536], pidx int32 [N] (pair-slot index of each
    keypoint's 2x2 patch), w f32 [4, N]"""
    fT = np.ascontiguousarray(
        np.asarray(feat_b, np.float32).reshape(C, H, W).transpose(1, 2, 0)
    ).astype(ml_dtypes.float8_e4m3)  # [H, W, C] fp8
    # part A: for pb in 0..31: for x: [row 2pb, row 2pb+1] -> [32, W, 2, C]
    partA = fT.reshape(H // 2, 2, W, C).transpose(0, 2, 1, 3)
    # part B: rows 1..62 -> for pb in 0..30: [row 2pb+1, row 2pb+2]
    partB = fT[1:H - 1].reshape((H - 2) // 2, 2, W, C).transpose(0, 2, 1, 3)
    featPair = np.concatenate([partA.reshape(-1), partB.reshape(-1)])
    x = np.asarray(kp_b[:, 0], np.float32)
    y = np.asarray(kp_b[:, 1], np.float32)
    x0 = np.minimum(np.floor(x), np.float32(W - 2)).astype(np.float32)
    y0 = np.minimum(np.floor(y), np.float32(H - 2)).astype(np.float32)
    wx = (x - x0).astype(np.float32)
    wy = (y - y0).astype(np.float32)
    x0i = x0.astype(np.int32)
    y0i = y0.astype(np.int32)
    even = (y0i % 2) == 0
    pidx = np.where(
        even,
        (y0i >> 1) * W + x0i,
        NPAIR_A + ((y0i - 1) >> 1) * W + x0i,
    ).astype(np.int32)
    w = np.stack(
        [(1 - wx) * (1 - wy), wx * (1 - wy), (1 - wx) * wy, wx * wy], 0
    ).astype(np.float32)
    return featPair, pidx, w


def _make_idx_layout(pidx):
    """[N] pair-slot indices -> [128, N/16] int16 SBUF index layout.
    Sequence order: per gather call q (256 idxs): [chunk 2q kps, chunk 2q+1
    kps]. Wrapped so sequence element i lives at [i%16 (replicated x8),
    i//16]."""
    seq = pidx.reshape(-1).astype(np.int32)  # already chunk-major
    lay = seq.reshape(-1, 16).T
    return np.tile(lay, (8, 1)).astype(np.int16)


def _make_wd(w1, w2):
    """weights [4,N] f32 per image -> [128, 64, 128] fp8 diagonal matrices"""
    wd = np.zeros((128, 2 * NCHUNK * 4, 128), np.float32)
    r = np.arange(128)
    for im, w in ((0, w1), (1, w2)):
        for ch in range(NCHUNK):
            for nb in range(4):
                k = (im * NCHUNK + ch) * 4 + nb
                wd[r, k, r] = w[nb, ch * 128:(ch + 1) * 128]
    return wd.astype(ml_dtypes.float8_e4m3)


def build_in_maps(feat1, feat2, kp1, kp2):
    in_maps = []
    for b in range(B):
        fP1, pi1, w1 = _host_prep_img(feat1[b], kp1[b])
        fP2, pi2, w2 = _host_prep_img(feat2[b], kp2[b])
        in_maps.append({
            "featP1": fP1,
            "featP2": fP2,
            "idx1": _make_idx_layout(pi1),
            "idx2": _make_idx_layout(pi2),
            "wd": _make_wd(w1, w2),
        })
    return in_maps


def kernel(feat1, feat2, kp1, kp2, kp1_mask, kp2_mask):
    from concourse.bass_utils import run_bass_kernel_spmd

    feat1 = np.asarray(feat1, np.float32)
    feat2 = np.asarray(feat2, np.float32)
    kp1 = np.asarray(kp1, np.float32)
    kp2 = np.asarray(kp2, np.float32)
    kp1_mask = np.asarray(kp1_mask)
    kp2_mask = np.asarray(kp2_mask)

    nc = get_nc()
    in_maps = build_in_maps(feat1, feat2, kp1, kp2)
    results = run_bass_kernel_spmd(nc, in_maps, list(range(B))).results

    sum_l2 = 0.0
    sum_valid = 0.0
    for b in range(B):
        r = results[b]["out_res"]
        n1sq = r[:, 0:8].T.reshape(-1).astype(np.float64)
        n2sq = r[:, 8:16].T.reshape(-1).astype(np.float64)
        dot = r[:, 16:24].T.reshape(-1).astype(np.float64)
        m1 = np.maximum(np.sqrt(n1sq), 1e-12)
        m2 = np.maximum(np.sqrt(n2sq), 1e-12)
        l2 = n1sq / (m1 * m1) + n2sq / (m2 * m2) - 2.0 * dot / (m1 * m2)
        valid = (kp1_mask[b] & kp2_mask[b]).astype(np.float64)
        sum_l2 += float((l2 * valid).sum())
        sum_valid += float(valid.sum())

    loss = 0.0 if sum_valid == 0 else sum_l2 / max(sum_valid, 1.0)
    return np.float32(loss)


# revision 15
# speedup vs baseline: 1.6695x; 1.0607x over previous
"""Trainium2 Bass kernel for KeypointAlignmentLossL2.

Strategy (data-parallel over batch, one NeuronCore per batch element):
  Host prep (per core b):
    - repack feat[b] into a pair-interleaved pixel-major fp8 layout
      ("featPair"): part A = rows (0,1),(2,3),... interleaved per column,
      part B = rows (1,2),(3,4),... . A keypoint's whole 2x2 bilinear patch
      is then 3072 contiguous bytes at a single host-computed index
      (part A for even y0, part B for odd y0) -> ONE gather descriptor per
      keypoint (SWDGE desc-gen at ~8 ns/desc is the pool-engine bottleneck).
    - bilinear weights packed as 128x128 fp8 diagonal matrices so the lerp
      runs on the tensor engine as accumulating diagonal matmuls
  Device (per core):
    - dma_gather (SWDGE prepare_only + trigger_dma so gathers pipeline
      back-to-back on the DMA engines), keypoint -> partition
    - TensorE: f = sum_nb diag(w_nb) @ g_nb accumulated in PSUM (f32),
      fp8 matmuls; the two N=384 halves land at psum[:, 128:512] and
      [:, 512:896] (each inside one bank, contiguous as a read region)
    - VectorE: copy f2 PSUM->SBUF bf16; scalar_tensor_tensor computes
      dot = sum(f1*f2) in one fused pass
    - ScalarE: activation(Square, accum_out) for |f1|^2 and |f2|^2
    - outputs one [128, 24] f32 tile (n1 | n2 | dot, keypoint-chunk layout)
  Host finish: masked mean of 2 - 2*cos distances across all cores.
"""
import copy as _pycopy
import numpy as np
import ml_dtypes

B, C, H, W, N = 8, 768, 64, 64, 1024
HW_ = H * W
NCHUNK = N // 128   # 8 chunks of 128 keypoints
NQ = 4              # gather calls per image; each covers 2 chunks (256 idxs)
NPAIR_A = HW_ // 2            # 2048 pair-slots in part A (even y0)
NPAIR_B = (H - 2) // 2 * W    # 1984 pair-slots in part B (odd y0)
NPAIR = NPAIR_A + NPAIR_B

_CACHE = {}


def _pair_ap(dram_handle):
    """Flat [NPAIR*1536] fp8 dram tensor -> AP [[1536, NPAIR-1], [1, 3072]]
    so dma_gather with elem_step=1536 and elem_size=3072 fetches a 2x2
    pixel patch per index (idx in pair-slot units)."""
    import bass_rust
    base = dram_handle[:].rearrange("(r c) -> r c", c=3072)
    ap = _pycopy.copy(base)
    ap.ap = bass_rust.VecI64Pair([[1536, NPAIR - 1], [1, 3072]])
    return ap


def _build_nc():
    from contextlib import ExitStack
    import concourse.bass as bass
    import concourse.tile as tile
    import concourse.mybir as mybir
    from concourse import bacc

    f32 = mybir.dt.float32
    bf16 = mybir.dt.bfloat16
    fp8 = mybir.dt.float8e4
    i16 = mybir.dt.int16
    MULT = mybir.AluOpType.mult
    SQUARE = mybir.ActivationFunctionType.Square

    nc = bacc.Bacc("TRN2", target_bir_lowering=False, debug=False, num_devices=8)

    featP1 = nc.dram_tensor("featP1", [NPAIR * 2 * C], fp8, kind="ExternalInput")
    featP2 = nc.dram_tensor("featP2", [NPAIR * 2 * C], fp8, kind="ExternalInput")
    idx1 = nc.dram_tensor("idx1", [128, N // 16], i16, kind="ExternalInput")
    idx2 = nc.dram_tensor("idx2", [128, N // 16], i16, kind="ExternalInput")
    wd = nc.dram_tensor("wd", [128, 2 * NCHUNK * 4, 128], fp8, kind="ExternalInput")
    out_res = nc.dram_tensor("out_res", [128, 3 * NCHUNK], f32, kind="ExternalOutput")

    feat_aps = (_pair_ap(featP1), _pair_ap(featP2))
    idxs_dram = (idx1, idx2)
    # corner nb (reference order: y0x0, y0x1, y1x0, y1x1) -> byte offset in
    # the gathered pair-interleaved patch [y0x0 | y1x0 | y0x1 | y1x1]
    CORNER_OFF = (0, 2 * C, C, 3 * C)

    with tile.TileContext(nc) as tc, ExitStack() as ctx:
        const_pool = ctx.enter_context(tc.tile_pool(name="const", bufs=1))
        f2c_pool = ctx.enter_context(tc.tile_pool(name="f2c", bufs=3))
        dump_pool = ctx.enter_context(tc.tile_pool(name="dump", bufs=6))
        ppool = ctx.enter_context(
            tc.tile_pool(name="p", bufs=4, space=bass.MemorySpace.PSUM)
        )

        # Explicit zero bias for activations: keeps bass's const-AP database
        # (pool-engine memsets) out of the kernel so the GPSIMD library load
        # for dma_gather starts immediately after the entry barrier.
        zbias = const_pool.tile([128, 1], f32, tag="zbias", name="zbias")
        nc.vector.memset(zbias[:], 0.0)

        wd_t = const_pool.tile([128, 2 * NCHUNK * 4, 128], fp8, tag="wd")
        nc.sync.dma_start(wd_t[:], wd[:])
        idx_t = []
        for im in range(2):
            t = const_pool.tile([128, N // 16], i16, tag=f"idx{im}", name=f"idx{im}")
            nc.sync.dma_start(t[:], idxs_dram[im][:])
            idx_t.append(t)

        # res layout: cols [0:8] = |f1|^2, [8:16] = |f2|^2, [16:24] = dot
        res = const_pool.tile([128, 3 * NCHUNK], f32, tag="res", name="res")

        # All gather outputs preallocated; preps+triggers issued up-front so
        # the SWDGE ring drains continuously. 4 calls of 512 idxs (1 image x
        # 4 chunks each): SWDGE desc-gen has ~0.6us fixed + tile adds a
        # ~1.4us IncSwdgeSem per prep, so fewer/bigger calls win.
        g_tiles = [[None, None] for _ in range(2)]  # [im][half]
        gsems = [[None, None] for _ in range(2)]
        for half in range(2):
            for im in range(2):
                g = const_pool.tile(
                    [128, 4, 4 * C], fp8, tag=f"g{im}_{half}", name=f"g{im}_{half}"
                )
                g_tiles[im][half] = g
                sem = nc.alloc_semaphore(f"gsem_{im}_{half}")
                gsems[im][half] = sem
                nc.gpsimd.dma_gather(
                    g[:],
                    feat_aps[im],
                    idx_t[im][:, half * 32:(half + 1) * 32],
                    512,
                    512,
                    4 * C,
                    elem_step=2 * C,
                    prepare_only=True,
                    sem=sem,
                )
                nc.gpsimd.trigger_dma(count=None)

        for ch in range(NCHUNK):
            half, r = ch // 4, ch % 4
            if True:
                # --- TensorE: bilinear lerp into PSUM, per image ---
                ps = []
                for im in range(2):
                    p = ppool.tile([128, 1024], f32, tag="ps")
                    for h in range(2):
                        for nb in range(4):
                            k = (im * NCHUNK + ch) * 4 + nb
                            off = CORNER_OFF[nb] + h * 384
                            mm = nc.tensor.matmul(
                                p[:, 128 + 384 * h:512 + 384 * h],
                                wd_t[:, k, :],
                                g_tiles[im][half][:, r, off:off + 384],
                                start=(nb == 0),
                                stop=(nb == 3),
                            )
                            if nb == 0:
                                # Gate each accumulation group on the gather's
                                # DMA-completion sem; tile's prepare_only path
                                # does not auto-gate on-chip consumers.
                                mm._wait_ge(gsems[im][half], 16)
                    ps.append(p)
                f1_ap = ps[0][:, 128:896]
                f2_ap = ps[1][:, 128:896]

                # --- VectorE: f2 PSUM->SBUF bf16, then fused dot ---
                f2c = f2c_pool.tile([128, 768], bf16, tag="f2c")
                nc.vector.tensor_copy(f2c[:], f2_ap)
                dump_d = dump_pool.tile([128, 768], bf16, tag="dump_d", name="dump_d")
                nc.vector.scalar_tensor_tensor(
                    dump_d[:], f1_ap, 1.0, f2c[:], MULT, MULT,
                    accum_out=res[:, 16 + ch:16 + ch + 1],
                )

                # --- ScalarE: |f1|^2 and |f2|^2 ---
                dump_a = dump_pool.tile([128, 768], bf16, tag="dump_a", name="dump_a")
                dump_b = dump_pool.tile([128, 768], bf16, tag="dump_b", name="dump_b")
                nc.scalar.activation(
                    dump_a[:], f1_ap, SQUARE, bias=zbias[:],
                    accum_out=res[:, ch:ch + 1],
                )
                nc.scalar.activation(
                    dump_b[:], f2c[:], SQUARE, bias=zbias[:],
                    accum_out=res[:, 8 + ch:8 + ch + 1],
                )

        nc.sync.dma_start(out_res[:], res[:])

    nc.compile()
    return nc


def get_nc():
    if "nc" not in _CACHE:
        _CACHE["nc"] = _build_nc()
    return _CACHE["nc"]


def _host_prep_img(feat_b, kp_b):
    """feat_b [C,H,W] f32, kp_b [N,2] f32 ->
    featPair fp8 flat [NPAIR*1536], pidx int32 [N] (pair-slot index of each
    keypoint's 2x2 patch), w f32 [4, N]"""
    fT = np.ascontiguousarray(
        np.asarray(feat_b, np.float32).reshape(C, H, W).transpose(1, 2, 0)
    ).astype(ml_dtypes.float8_e4m3)  # [H, W, C] fp8
    # part A: for pb in 0..31: for x: [row 2pb, row 2pb+1] -> [32, W, 2, C]
    partA = fT.reshape(H // 2, 2, W, C).transpose(0, 2, 1, 3)
    # part B: rows 1..62 -> for pb in 0..30: [row 2pb+1, row 2pb+2]
    partB = fT[1:H - 1].reshape((H - 2) // 2, 2, W, C).transpose(0, 2, 1, 3)
    featPair = np.concatenate([partA.reshape(-1), partB.reshape(-1)])
    x = np.asarray(kp_b[:, 0], np.float32)
    y = np.asarray(kp_b[:, 1], np.float32)
    x0 = np.minimum(np.floor(x), np.float32(W - 2)).astype(np.float32)
    y0 = np.minimum(np.floor(y), np.float32(H - 2)).astype(np.float32)
    wx = (x - x0).astype(np.float32)
    wy = (y - y0).astype(np.float32)
    x0i = x0.astype(np.int32)
    y0i = y0.astype(np.int32)
    even = (y0i % 2) == 0
    pidx = np.where(
        even,
        (y0i >> 1) * W + x0i,
        NPAIR_A + ((y0i - 1) >> 1) * W + x0i,
    ).astype(np.int32)
    w = np.stack(
        [(1 - wx) * (1 - wy), wx * (1 - wy), (1 - wx) * wy, wx * wy], 0
    ).astype(np.float32)
    return featPair, pidx, w


def _make_idx_layout(pidx):
    """[N] pair-slot indices -> [128, N/16] int16 SBUF index layout.
    Sequence order: per gather call q (256 idxs): [chunk 2q kps, chunk 2q+1
    kps]. Wrapped so sequence element i lives at [i%16 (replicated x8),
    i//16]."""
    seq = pidx.reshape(-1).astype(np.int32)  # already chunk-major
    lay = seq.reshape(-1, 16).T
    return np.tile(lay, (8, 1)).astype(np.int16)


def _make_wd(w1, w2):
    """weights [4,N] f32 per image -> [128, 64, 128] fp8 diagonal matrices"""
    wd = np.zeros((128, 2 * NCHUNK * 4, 128), np.float32)
    r = np.arange(128)
    for im, w in ((0, w1), (1, w2)):
        for ch in range(NCHUNK):
            for nb in range(4):
                k = (im * NCHUNK + ch) * 4 + nb
                wd[r, k, r] = w[nb, ch * 128:(ch + 1) * 128]
    return wd.astype(ml_dtypes.float8_e4m3)


def build_in_maps(feat1, feat2, kp1, kp2):
    in_maps = []
    for b in range(B):
        fP1, pi1, w1 = _host_prep_img(feat1[b], kp1[b])
        fP2, pi2, w2 = _host_prep_img(feat2[b], kp2[b])
        in_maps.append({
            "featP1": fP1,
            "featP2": fP2,
            "idx1": _make_idx_layout(pi1),
            "idx2": _make_idx_layout(pi2),
            "wd": _make_wd(w1, w2),
        })
    return in_maps


def kernel(feat1, feat2, kp1, kp2, kp1_mask, kp2_mask):
    from concourse.bass_utils import run_bass_kernel_spmd

    feat1 = np.asarray(feat1, np.float32)
    feat2 = np.asarray(feat2, np.float32)
    kp1 = np.asarray(kp1, np.float32)
    kp2 = np.asarray(kp2, np.float32)
    kp1_mask = np.asarray(kp1_mask)
    kp2_mask = np.asarray(kp2_mask)

    nc = get_nc()
    in_maps = build_in_maps(feat1, feat2, kp1, kp2)
    results = run_bass_kernel_spmd(nc, in_maps, list(range(B))).results

    sum_l2 = 0.0
    sum_valid = 0.0
    for b in range(B):
        r = results[b]["out_res"]
        n1sq = r[:, 0:8].T.reshape(-1).astype(np.float64)
        n2sq = r[:, 8:16].T.reshape(-1).astype(np.float64)
        dot = r[:, 16:24].T.reshape(-1).astype(np.float64)
        m1 = np.maximum(np.sqrt(n1sq), 1e-12)
        m2 = np.maximum(np.sqrt(n2sq), 1e-12)
        l2 = n1sq / (m1 * m1) + n2sq / (m2 * m2) - 2.0 * dot / (m1 * m2)
        valid = (kp1_mask[b] & kp2_mask[b]).astype(np.float64)
        sum_l2 += float((l2 * valid).sum())
        sum_valid += float(valid.sum())

    loss = 0.0 if sum_valid == 0 else sum_l2 / max(sum_valid, 1.0)
    return np.float32(loss)
